# revision 3
# baseline (speedup 1.0000x reference)
"""Causal self-attention (B=4, T=2048, C=1024, H=16) on 8 NeuronCores.

Sharding: core c handles batch b = c//2 and head-half half = c%2 (8 heads,
512 channels). QKV projections are column-parallel, output projection is
row-parallel (Megatron); the two per-batch bf16 output partials are summed
on host in fp32.

All matmul operands are bf16 (fp32 PSUM accumulate): same 1 cycle/row PE
stream rate as fp32r but with fast-weight-load, 2-4x DVE element ops, and
half the DMA/SBUF footprint. Tolerance is 2e-2; bf16 end-to-end measures
~1e-3.

Per-core pipeline:
  phase 1: qT/kT = W @ xT (weight-stationary, [m, t] layout, bias applied
           for free on the ACT psum-evacuation via per-partition bias),
           v = xT.T @ WvT ([t, m] layout) with a constant ones-column per
           head (vAug) so the attention matmul also produces the softmax
           denominator.
  phase 2: per 1024-wide query chunk, per head: P = exp(S^T) block by block
           (128-row key tiles), 0/1 mask multiply on the diagonal block,
           y_psum[65, 1024] += vAug^T @ P accumulated over key tiles.
           Softmax uses a fixed max of 0 (scores are ~N(0,1)); denominator
           comes out of the matmul (row 64). Normalize via reciprocal +
           ones-broadcast + multiply, writing yT straight into SBUF.
  phase 3: out_partial = yT.T @ WpT (+ bias on half==0 cores only), bf16.
"""

import sys
import types

import numpy as np
from contextlib import ExitStack

import ml_dtypes

import concourse.bass as bass
import concourse.mybir as mybir
import concourse.tile as tile
from concourse import bacc
from concourse.bass_utils import run_bass_kernel_spmd

# If the environment sets BASS_TRACE but ships only the antenv stub (no
# axon_hooks), run_bass_kernel_spmd would crash on import. Provide the
# graceful "no hook registered" fallback only when the real module is absent.
try:  # pragma: no cover
    import antenv.axon_hooks  # noqa: F401
except ImportError:  # pragma: no cover
    import antenv

    _stub = types.ModuleType("antenv.axon_hooks")
    _stub.get_axon_ntff_profile_hook = lambda: None
    sys.modules["antenv.axon_hooks"] = _stub
    antenv.axon_hooks = _stub

F32 = mybir.dt.float32
BF16 = mybir.dt.bfloat16
EXP = mybir.ActivationFunctionType.Exp
IDENT = mybir.ActivationFunctionType.Identity
NP_BF16 = ml_dtypes.bfloat16

B, T, C, H = 4, 2048, 1024, 16
HD = C // H              # 64 head dim
N_CORES = 8
HPC = H // 2             # 8 heads per core
MPC = C // 2             # 512 channels per core
MT = MPC // 128          # 4 m-tiles per core
CT = C // 128            # 8 contraction tiles
TC = T // 512            # 4 t-chunks
TT = T // 128            # 16 t-tiles
SCALE = float(1.0 / np.sqrt(HD))

_CACHE = {}


def _build(bench_loops=None, phases=(1, 2, 3)):
    import contextlib

    def _rep(tc):
        # bench builds repeat each phase body in a hardware For_i loop
        return tc.For_i(0, bench_loops, 1) if bench_loops else contextlib.nullcontext()

    nc = bacc.Bacc()
    xT = nc.declare_dram_parameter("xT", [C, T], BF16, isOutput=False)
    wqT = nc.declare_dram_parameter("wqT", [C, MPC], BF16, isOutput=False)
    wkT = nc.declare_dram_parameter("wkT", [C, MPC], BF16, isOutput=False)
    wvT = nc.declare_dram_parameter("wvT", [C, MPC], BF16, isOutput=False)
    wpT = nc.declare_dram_parameter("wpT", [MPC, C], BF16, isOutput=False)
    bqc = nc.declare_dram_parameter("bqc", [128, MT], BF16, isOutput=False)
    bkc = nc.declare_dram_parameter("bkc", [128, MT], BF16, isOutput=False)
    bv = nc.declare_dram_parameter("bv", [1, MPC], BF16, isOutput=False)
    bp = nc.declare_dram_parameter("bp", [1, C], BF16, isOutput=False)
    mask01d = nc.declare_dram_parameter("mask01", [128, 128], BF16, isOutput=False)
    outp = nc.declare_dram_parameter("out", [T, C], BF16, isOutput=True)

    with tile.TileContext(nc) as tc:
        with ExitStack() as ctx:
            persist = ctx.enter_context(tc.tile_pool(name="persist", bufs=1))
            pool_p = ctx.enter_context(tc.tile_pool(name="pool_p", bufs=2))
            ps_big = ctx.enter_context(tc.tile_pool(name="ps_big", bufs=2, space="PSUM"))
            ps_y = ctx.enter_context(tc.tile_pool(name="ps_y", bufs=2, space="PSUM"))

            # ---- constants / small tensors ----
            mask01_sb = persist.tile([128, 128], BF16, name="mask01_sb")
            nc.sync.dma_start(out=mask01_sb, in_=mask01d[:, :])
            bqc_sb = persist.tile([128, MT], BF16, name="bqc_sb")
            nc.sync.dma_start(out=bqc_sb, in_=bqc[:, :])
            bkc_sb = persist.tile([128, MT], BF16, name="bkc_sb")
            nc.sync.dma_start(out=bkc_sb, in_=bkc[:, :])
            bv_sb = persist.tile([1, MPC], BF16, name="bv_sb")
            nc.sync.dma_start(out=bv_sb, in_=bv[:, :])
            bp_sb = persist.tile([1, C], BF16, name="bp_sb")
            nc.sync.dma_start(out=bp_sb, in_=bp[:, :])
            ones_t = persist.tile([1, 128], BF16, name="ones_t")
            nc.vector.memset(ones_t, 1.0)
            ones8 = persist.tile([128, 8], BF16, name="ones8")
            nc.vector.memset(ones8, 1.0)

            # ---- persistent activations ----
            qT_sb = [persist.tile([128, T], BF16, name=f"qT{m}") for m in range(MT)]
            kT_sb = [persist.tile([128, T], BF16, name=f"kT{m}") for m in range(MT)]
            vAug = [persist.tile([128, HPC * (HD + 1)], BF16, name=f"vAug{t}") for t in range(TT)]
            yT_sb = [persist.tile([128, T], BF16, name=f"yT{m}") for m in range(MT)]

            # constant ones-column of vAug, written once
            for t_ in range(TT):
                va = vAug[t_].rearrange("p (h w) -> p h w", w=HD + 1)
                nc.vector.tensor_copy(va[:, :, HD], ones8)

            if 1 not in phases:
                # bench-only: initialize phase-1 products with arbitrary finite data
                for m in range(MT):
                    nc.sync.dma_start(out=qT_sb[m], in_=xT[0:128, :])
                    nc.sync.dma_start(out=kT_sb[m], in_=xT[128:256, :])
                for t_ in range(TT):
                    va = vAug[t_].rearrange("p (h w) -> p h w", w=HD + 1)
                    nc.sync.dma_start(out=va[:, :, 0:HD],
                                      in_=xT[0:128, 0:HPC * HD].rearrange("p (h w) -> p h w", w=HD))
            if 3 in phases and 2 not in phases:
                for m in range(MT):
                    nc.sync.dma_start(out=yT_sb[m], in_=xT[0:128, :])

            # ================= phase 1: projections =================
            with ExitStack() as ctx1:
              if 1 in phases:
                  pool_w = ctx1.enter_context(tc.tile_pool(name="pool_w", bufs=1))
                  pool_xs = ctx1.enter_context(tc.tile_pool(name="pool_xs", bufs=1))
                  wq_t = [pool_w.tile([128, MPC], BF16, name=f"wq{c}") for c in range(CT)]
                  wk_t = [pool_w.tile([128, MPC], BF16, name=f"wk{c}") for c in range(CT)]
                  wv_t = [pool_w.tile([128, MPC], BF16, name=f"wv{c}") for c in range(CT)]
                  for c in range(CT):
                      nc.sync.dma_start(out=wq_t[c], in_=wqT[c * 128:(c + 1) * 128, :])
                      nc.sync.dma_start(out=wk_t[c], in_=wkT[c * 128:(c + 1) * 128, :])
                      nc.sync.dma_start(out=wv_t[c], in_=wvT[c * 128:(c + 1) * 128, :])

                  rep1 = ctx1.enter_context(_rep(tc))
                  for tch in range(TC):
                      t0 = tch * 512
                      xs = []
                      for c in range(CT):
                          x_ = pool_xs.tile([128, 512], BF16, name=f"xs_{tch}_{c}", tag="xs", bufs=12)
                          nc.sync.dma_start(out=x_, in_=xT[c * 128:(c + 1) * 128, t0:t0 + 512])
                          xs.append(x_)
                      # qT / kT (weight-stationary): psum[m 128, t 512]; bias is
                      # per-partition here so the ACT evacuation applies it free.
                      for wt, bias_col, dst in ((wq_t, bqc_sb, qT_sb), (wk_t, bkc_sb, kT_sb)):
                          for m in range(MT):
                              ps = ps_big.tile([128, 512], F32, name=f"ps_{tch}_{id(wt)}_{m}", tag="big", padded_shape=[128, 1024])
                              for c in range(CT):
                                  nc.tensor.matmul(ps, wt[c][:, m * 128:(m + 1) * 128], xs[c],
                                                   start=(c == 0), stop=(c == CT - 1))
                              nc.scalar.activation(out=dst[m][:, t0:t0 + 512], in_=ps,
                                                   func=IDENT, bias=bias_col[:, m:m + 1])
                      # v (x-stationary): psum[t 128, m 512] -> vAug
                      for tt in range(4):
                          tg = tch * 4 + tt
                          ps = ps_big.tile([128, MPC], F32, name=f"psv_{tg}", tag="big", padded_shape=[128, 1024])
                          for c in range(CT):
                              nc.tensor.matmul(ps, xs[c][:, tt * 128:(tt + 1) * 128], wv_t[c],
                                               start=(c == 0), stop=False)
                          nc.tensor.matmul(ps, ones_t, bv_sb, start=False, stop=True)
                          va = vAug[tg].rearrange("p (h w) -> p h w", w=HD + 1)
                          nc.vector.tensor_copy(va[:, :, 0:HD],
                                                ps.rearrange("p (h w) -> p h w", w=HD))

            # ================= phase 2: attention =================
            with ExitStack() as ctx2:
              pool_p2 = ctx2.enter_context(tc.tile_pool(name="pool_p2", bufs=3))
              ctx2.enter_context(_rep(tc) if 2 in phases else __import__("contextlib").nullcontext())
              if 2 in phases:
                  for ic in range(2):
                      i0 = ic * 1024
                      jmax = 8 * ic + 7
                      for h in range(HPC):
                          mt, so = h // 2, (h % 2) * 64
                          kT_h = kT_sb[mt]
                          qT_h = qT_sb[mt]
                          psy = ps_y.tile([HD + 1, 1024], F32, name=f"psy_{h}_{ic}", tag="psy")
                          # software-pipelined emission: y(j-1) is emitted after
                          # S(j)/exp(j) so the PE fills exp latency with the next
                          # score block instead of stalling.
                          pend = None

                          def _emit_y(j, P):
                              lo = max(0, j * 128 - i0)
                              for a, b in ((lo, 512), (max(lo, 512), 1024)):
                                  if a < b:
                                      nc.tensor.matmul(psy[:, a:b],
                                                       vAug[j][:, h * (HD + 1):(h + 1) * (HD + 1)],
                                                       P[:, a:b],
                                                       start=(j == 0), stop=(j == jmax))

                          for j in range(jmax + 1):
                              lo = max(0, j * 128 - i0)
                              ps_s = ps_big.tile([128, 1024], F32, name=f"pss_{h}_{ic}_{j}", tag="big")
                              for a, b in ((lo, 512), (max(lo, 512), 1024)):
                                  if a < b:
                                      nc.tensor.matmul(ps_s[:, a:b],
                                                       kT_h[so:so + HD, j * 128:(j + 1) * 128],
                                                       qT_h[so:so + HD, i0 + a:i0 + b],
                                                       start=True, stop=True)
                              P = pool_p.tile([128, 1024], BF16, name=f"P_{h}_{ic}_{j}", tag="P", bufs=3)
                              nc.scalar.activation(out=P[:, lo:1024], in_=ps_s[:, lo:1024],
                                                   func=EXP, scale=SCALE)
                              if j * 128 >= i0:  # diagonal block: in-tile causal mask
                                  nc.vector.tensor_mul(P[:, lo:lo + 128], P[:, lo:lo + 128], mask01_sb)
                              if pend is not None:
                                  _emit_y(*pend)
                              pend = (j, P)
                          _emit_y(*pend)
                          # decouple the tail: one copy frees the psum accumulator,
                          # the normalize chain then runs off the critical path.
                          yu = pool_p2.tile([HD + 1, 1024], BF16, name=f"yu_{h}_{ic}", tag="yu")
                          nc.vector.tensor_copy(yu, psy)
                          r16 = pool_p2.tile([1, 1024], BF16, name=f"r16_{h}_{ic}", tag="r16")
                          with nc.allow_low_precision(reason="denominator recip; 2e-2 tol"):
                              nc.vector.reciprocal(r16, yu[HD:HD + 1, :])
                          rb = pool_p2.tile([HD, 1024], BF16, name=f"rb_{h}_{ic}", tag="rb")
                          nc.gpsimd.partition_broadcast(rb, r16)
                          nc.vector.tensor_mul(yT_sb[mt][so:so + HD, i0:i0 + 1024],
                                               yu[0:HD, :], rb)

            # ================= phase 3: output projection =================
            with ExitStack() as ctx3:
              if 3 in phases:
                  pool_3 = ctx3.enter_context(tc.tile_pool(name="pool_3", bufs=1))
                  wp_t = [pool_3.tile([128, C], BF16, name=f"wp{m}") for m in range(MT)]
                  for m in range(MT):
                      nc.sync.dma_start(out=wp_t[m], in_=wpT[m * 128:(m + 1) * 128, :])
                  rep3 = ctx3.enter_context(_rep(tc))
                  for tt in range(TT):
                      for nch in range(2):
                          n0 = nch * 512
                          ps = ps_big.tile([128, 512], F32, name=f"pso_{tt}_{nch}", tag="big", padded_shape=[128, 1024])
                          for m in range(MT):
                              nc.tensor.matmul(ps, yT_sb[m][:, tt * 128:(tt + 1) * 128],
                                               wp_t[m][:, n0:n0 + 512],
                                               start=(m == 0), stop=False)
                          nc.tensor.matmul(ps, ones_t, bp_sb[:, n0:n0 + 512],
                                           start=False, stop=True)
                          o_sb = pool_3.tile([128, 512], BF16, name=f"o_{tt}_{nch}", tag="o", bufs=3)
                          nc.scalar.copy(o_sb, ps)
                          nc.sync.dma_start(out=outp[tt * 128:(tt + 1) * 128, n0:n0 + 512], in_=o_sb)
    nc.finalize()
    return nc


def _get_nc(bench_loops=None, phases=(1, 2, 3)):
    key = ("nc", bench_loops, tuple(phases))
    if key not in _CACHE:
        _CACHE[key] = _build(bench_loops, phases)
    return _CACHE[key]


def make_in_maps(x, Wk, bk, Wq, bq, Wv, bv, Wp, bp):
    x = np.asarray(x, dtype=np.float32)
    Wk, Wq, Wv, Wp = (np.asarray(a, dtype=np.float32) for a in (Wk, Wq, Wv, Wp))
    bk, bq, bv, bp = (np.asarray(a, dtype=np.float32) for a in (bk, bq, bv, bp))

    mask01 = np.where(np.tril(np.ones((128, 128), dtype=bool)).T, 1.0, 0.0).astype(NP_BF16)
    xT_b = [np.ascontiguousarray(x[b].T).astype(NP_BF16) for b in range(B)]
    in_maps = []
    for c in range(N_CORES):
        b, half = c // 2, c % 2
        hs = half * MPC
        in_maps.append({
            "xT": xT_b[b],
            "wqT": np.ascontiguousarray(Wq[hs:hs + MPC, :].T).astype(NP_BF16),
            "wkT": np.ascontiguousarray(Wk[hs:hs + MPC, :].T).astype(NP_BF16),
            "wvT": np.ascontiguousarray(Wv[hs:hs + MPC, :].T).astype(NP_BF16),
            "wpT": np.ascontiguousarray(Wp[:, hs:hs + MPC].T).astype(NP_BF16),
            # per-partition column layout for the ACT-side bias
            "bqc": np.ascontiguousarray(bq[hs:hs + MPC].reshape(MT, 128).T).astype(NP_BF16),
            "bkc": np.ascontiguousarray(bk[hs:hs + MPC].reshape(MT, 128).T).astype(NP_BF16),
            "bv": bv[hs:hs + MPC].reshape(1, MPC).astype(NP_BF16),
            "bp": (bp if half == 0 else np.zeros_like(bp)).reshape(1, C).astype(NP_BF16),
            "mask01": mask01,
        })
    return in_maps


def kernel(x, Wk, bk, Wq, bq, Wv, bv, Wp, bp, **run_kwargs):
    in_maps = make_in_maps(x, Wk, bk, Wq, bq, Wv, bv, Wp, bp)
    nc = _get_nc()
    res = run_bass_kernel_spmd(nc, in_maps, core_ids=list(range(N_CORES)), **run_kwargs)
    out = np.empty((B, T, C), dtype=np.float32)
    for b in range(B):
        out[b] = (res.results[2 * b]["out"].astype(np.float32)
                  + res.results[2 * b + 1]["out"].astype(np.float32))
    if run_kwargs:
        kernel.last_results = res
    return out
